# revision 2
# baseline (speedup 1.0000x reference)
"""LIF spiking-neuron (soft reset) Bass kernel for Trainium2, 8-core SPMD.

Input  x: [B=32, C=128, T=16, H=32, W=32] f32
Output s: same shape, spikes in {0, 1}.

Recurrence per element over T:
    m' = z * 0.75 + x_t              (integrate; z = post-reset membrane)
    s_t = (m' > 0.5)                 (spike)
    z   = m' - 0.5 * s_t             (soft reset)

Engine assignment (the point of this design): on TRN2 the DVE and GpSimd
engines arbitrate an exclusive shared SBUF port pair, so 2-src DVE ops and
any GpSimd op serialize against each other. We spread the per-step work over
engines with private ports:

    DVE    : m'_g = stt(z_g(PSUM) * beta + x_t,g)      -> SBUF   (1 op/group)
    GpSimd : s_g  = (m'_g > 0.5)  -> fp8e4 {0,1}                 (3 of 4 groups)
    DVE    : same, for the remaining group
    PE     : z_g(PSUM) = I_f32 @ m'_g  +  (-0.5 I_fp8) @ s_g     (exact: every
             product is value*1.0 or {0,1}*-0.5; fp32 accumulate; verified
             bit-exact on HW)
    DMA    : s_g shipped as raw fp8 bytes (1B/elem); host maps to f32 {0,1}

The spike tiles double as both the PE's reset operand and the DMA output, so
each element costs one integrate, one compare, and four matmul passes. All
f32 roundings match the reference order bit-for-bit (rel err 0).

Sharding: batch dim split across 8 cores (4 per core); per core the shard is
[512 (b*c) rows, 16 t, 1024 hw], rows mapped to SBUF partitions in 4 groups
of 128.
"""

import numpy as np

import concourse.bacc as bacc
import concourse.mybir as mybir
import concourse.tile as tile
from concourse.bass_utils import run_bass_kernel_spmd

B, C, T, H, W = 32, 128, 16, 32, 32
NCORES = 8
B_PER = B // NCORES          # 4
ROWS = B_PER * C             # 512
HW = H * W                   # 1024
P = 128
NG = ROWS // P               # 4 partition groups
BETA = 0.75
THRESH = 0.5

F32 = mybir.dt.float32
FP8 = mybir.dt.float8e4
ALU = mybir.AluOpType

# Which engine computes the spike compare per group: "g" = GpSimd, "v" = DVE.
SPIKE_ENG = ("v", "g", "g", "g")

_nc_cache = None


def _build():
    import ml_dtypes

    nc = bacc.Bacc(
        "TRN2",
        target_bir_lowering=False,
        debug=False,
        enable_asserts=False,
        num_devices=NCORES,
    )
    x_d = nc.dram_tensor("x", [ROWS, T, HW], F32, kind="ExternalInput").ap()
    s_d = nc.dram_tensor("s", [ROWS, T, HW], FP8, kind="ExternalOutput").ap()

    x_v = x_d.rearrange("(g p) t f -> g p t f", p=P)
    s_v = s_d.rearrange("(g p) t f -> g p t f", p=P)

    eye = np.eye(P, dtype=np.float32)
    wi_d = nc.inline_tensor(eye, name="wi")
    wn8_d = nc.inline_tensor(
        (-THRESH * eye).astype(ml_dtypes.float8_e4m3fn), name="wn8")

    HALF = HW // 2  # fp32 matmul moving-operand cap / one PSUM bank

    with tile.TileContext(nc) as tc:
        with (
            tc.tile_pool(name="mp", bufs=1) as m_pool,
            tc.tile_pool(name="xp", bufs=3) as x_pool,
            tc.tile_pool(name="sp", bufs=3) as s_pool,
            tc.tile_pool(name="wp", bufs=1) as w_pool,
            tc.tile_pool(name="zp", bufs=1, space="PSUM") as z_pool,
        ):
            wi = w_pool.tile([P, P], F32, tag="wi", name="wi")
            wn8 = w_pool.tile([P, P], FP8, tag="wn8", name="wn8")
            nc.sync.dma_start(wi[:], wi_d.ap()[:])
            nc.sync.dma_start(wn8[:], wn8_d.ap()[:])

            z_tiles = [
                z_pool.tile([P, HW], F32, tag=f"z{g}", name=f"z{g}")
                for g in range(NG)
            ]
            m_tiles = [
                m_pool.tile([P, HW], F32, tag=f"m{g}", name=f"m{g}")
                for g in range(NG)
            ]

            def load_x(t, split=False):
                xt = x_pool.tile([P, NG, HW], F32, tag="xt", name="xt")
                if split:
                    for g in range(NG):
                        nc.sync.dma_start(xt[:, g, :], x_v[g, :, t, :])
                else:
                    nc.sync.dma_start(
                        xt[:], x_v[:, :, t, :].rearrange("g p f -> p g f")
                    )
                return xt

            def spike(dst, src, eng):
                if eng == "g":
                    nc.gpsimd.tensor_scalar(dst, src, THRESH, None, ALU.is_gt)
                else:
                    nc.vector.tensor_scalar(dst, src, THRESH, None, ALU.is_gt)

            x0 = load_x(0, split=True)
            xt = x0
            for t in range(T):
                x_next = load_x(t + 1) if t < T - 1 else None
                srcs = []
                for g in range(NG):
                    if t == 0:
                        srcs.append(x0[:, g, :])
                    else:
                        nc.vector.scalar_tensor_tensor(
                            m_tiles[g][:], z_tiles[g][:], BETA, xt[:, g, :],
                            op0=ALU.mult, op1=ALU.add,
                        )
                        srcs.append(m_tiles[g][:])
                s = s_pool.tile([P, NG, HW], FP8, tag="st", name="st")
                for g in range(NG):
                    spike(s[:, g, :], srcs[g], SPIKE_ENG[g])
                nc.scalar.dma_start(
                    s_v[:, :, t, :].rearrange("g p f -> p g f"), s[:]
                )
                if t < T - 1:
                    for g in range(NG):
                        for c in range(2):
                            cols = slice(c * HALF, (c + 1) * HALF)
                            nc.tensor.matmul(
                                z_tiles[g][:, cols], wi[:], srcs[g][:, cols],
                                start=True, stop=False,
                            )
                        for c in range(2):
                            cols = slice(c * HALF, (c + 1) * HALF)
                            nc.tensor.matmul(
                                z_tiles[g][:, cols], wn8[:], s[:, g, cols],
                                start=False, stop=True,
                            )
                xt = x_next
    nc.compile()
    return nc


def _get_nc():
    global _nc_cache
    if _nc_cache is None:
        _nc_cache = _build()
    return _nc_cache


def _run(x, **spmd_kwargs):
    x = np.ascontiguousarray(np.asarray(x, dtype=np.float32))
    assert x.shape == (B, C, T, H, W)
    nc = _get_nc()
    in_maps = [
        {"x": x[i * B_PER:(i + 1) * B_PER].reshape(ROWS, T, HW)}
        for i in range(NCORES)
    ]
    res = run_bass_kernel_spmd(nc, in_maps, list(range(NCORES)), **spmd_kwargs)
    out = np.concatenate(
        [
            np.asarray(r["s"]).astype(np.float32).reshape(B_PER, C, T, H, W)
            for r in res.results
        ],
        axis=0,
    )
    return out, res


def kernel(x):
    out, _ = _run(x)
    return out


# revision 7
# speedup vs baseline: 5.3627x; 5.3627x over previous
"""LIF spiking-neuron (soft reset) Bass kernel for Trainium2, 8-core SPMD.

Input  x: [B=32, C=128, T=16, H=32, W=32] f32
Output s: same shape, spikes in {0, 1}.

Recurrence per element over T:
    m' = z * 0.75 + x_t              (integrate; z = post-reset membrane)
    s_t = (m' > 0.5)                 (spike)
    z   = m' - 0.5 * s_t             (soft reset)

Engine assignment (the point of this design): on TRN2, DVE and GpSimd
arbitrate an exclusive shared SBUF port pair, so GpSimd buys nothing while
DVE runs 2-src ops back-to-back; and every op here except the spike compare
is unavailable or slow elsewhere. The schedule keeps only two ops per
element on the DVE and farms the reset out to the private-port engines:

    DVE : m'_g(SBUF) = stt(z_g(PSUM) * beta + x_t,g)         ~1.25 us
    DVE : s_g(SBUF)  = (m'_g > 0.5) -> fp8e4 {0,1}           ~0.7 us
    ACT : copy m'_g -> z_g (PSUM overwrite; private ports)   ~1.0 us
    PE  : z_g += (-0.5 I_fp8) @ s_g  (matmul accumulate onto the ACT-written
          value; start=False adds unconditionally - HW-verified)
    DMA : s_g shipped as raw fp8 bytes (1B/elem); host maps to f32 {0,1}

All f32 roundings match the reference order bit-for-bit (the fp8 matmul
products are 1.0*-0.5 or 0.0, and m' - 0.5*s is exactly representable), so
rel err is 0. The spike tile doubles as the PE operand and the DMA output.

Sharding: batch dim split across 8 cores (4 per core); per core the shard is
[512 (b*c) rows, 16 t, 1024 hw], rows mapped to SBUF partitions in 4 groups
of 128. z lives in PSUM: 4 groups x 4KB = all 8 banks.
"""

import numpy as np

import concourse.bacc as bacc
import concourse.mybir as mybir
import concourse.tile as tile
from concourse.bass_utils import run_bass_kernel_spmd

B, C, T, H, W = 32, 128, 16, 32, 32
NCORES = 8
B_PER = B // NCORES          # 4
ROWS = B_PER * C             # 512
HW = H * W                   # 1024
P = 128
NG = ROWS // P               # 4 partition groups
BETA = 0.75
THRESH = 0.5

F32 = mybir.dt.float32
FP8 = mybir.dt.float8e4
ALU = mybir.AluOpType

_nc_cache = None


def _build():
    import ml_dtypes

    nc = bacc.Bacc(
        "TRN2",
        target_bir_lowering=False,
        debug=False,
        enable_asserts=False,
        num_devices=NCORES,
    )
    x_d = nc.dram_tensor("x", [ROWS, T, HW], F32, kind="ExternalInput").ap()
    s_d = nc.dram_tensor("s", [ROWS, T, HW], FP8, kind="ExternalOutput").ap()

    x_v = x_d.rearrange("(g p) t f -> g p t f", p=P)
    s_v = s_d.rearrange("(g p) t f -> g p t f", p=P)

    wn8_d = nc.inline_tensor(
        (-THRESH * np.eye(P)).astype(ml_dtypes.float8_e4m3fn), name="wn8")
    wz_d = nc.inline_tensor(
        np.zeros((P, P), dtype=ml_dtypes.float8_e4m3fn), name="wz")

    HALF = HW // 2  # one PSUM bank of fp32

    with tile.TileContext(nc) as tc:
        with (
            tc.tile_pool(name="mp", bufs=1) as m_pool,
            tc.tile_pool(name="xp", bufs=3) as x_pool,
            tc.tile_pool(name="sp", bufs=3) as s_pool,
            tc.tile_pool(name="wp", bufs=1) as w_pool,
            tc.tile_pool(name="zp", bufs=1, space="PSUM") as z_pool,
        ):
            wn8 = w_pool.tile([P, P], FP8, tag="wn8", name="wn8")
            wz = w_pool.tile([P, P], FP8, tag="wz", name="wz")
            nc.sync.dma_start(wn8[:], wn8_d.ap()[:])
            nc.sync.dma_start(wz[:], wz_d.ap()[:])

            z_tiles = [
                z_pool.tile([P, HW], F32, tag=f"z{g}", name=f"z{g}")
                for g in range(NG)
            ]
            m_tiles = [
                m_pool.tile([P, HW], F32, tag=f"m{g}", name=f"m{g}")
                for g in range(NG)
            ]

            def load_x(t, split=False):
                xt = x_pool.tile([P, NG, HW], F32, tag="xt", name="xt")
                if split:
                    for g in range(NG):
                        nc.sync.dma_start(xt[:, g, :], x_v[g, :, t, :])
                else:
                    nc.sync.dma_start(
                        xt[:], x_v[:, :, t, :].rearrange("g p f -> p g f")
                    )
                return xt

            x0 = load_x(0)
            xt = x0
            for t in range(T):
                x_next = load_x(t + 1) if t < T - 1 else None
                srcs = []
                for g in range(NG):
                    if t == 0:
                        srcs.append(x0[:, g, :])
                    else:
                        nc.vector.scalar_tensor_tensor(
                            m_tiles[g][:], z_tiles[g][:], BETA, xt[:, g, :],
                            op0=ALU.mult, op1=ALU.add,
                        )
                        srcs.append(m_tiles[g][:])
                s = s_pool.tile([P, NG, HW], FP8, tag="st", name="st")
                for g in range(NG):
                    nc.vector.tensor_scalar(
                        s[:, g, :], srcs[g], THRESH, None, ALU.is_gt)
                nc.scalar.dma_start(
                    s_v[:, :, t, :].rearrange("g p f -> p g f"), s[:]
                )
                if t < T - 1:
                    for g in range(NG):
                        if t == 0:
                            # A start=False matmul on a bank whose per-element
                            # has_written bits are clear OVERWRITES instead of
                            # accumulating. Set the bits (and zero the bank)
                            # with a 0-weight pass before the ACT copy.
                            for c in range(2):
                                cols = slice(c * HALF, (c + 1) * HALF)
                                nc.tensor.matmul(
                                    z_tiles[g][:, cols], wz[:], s[:, g, cols],
                                    start=True, stop=False,
                                    skip_group_check=True,
                                )
                        nc.scalar.copy(z_tiles[g][:], srcs[g])
                        for c in range(2):
                            cols = slice(c * HALF, (c + 1) * HALF)
                            nc.tensor.matmul(
                                z_tiles[g][:, cols], wn8[:], s[:, g, cols],
                                start=False, stop=True, skip_group_check=True,
                            )
                xt = x_next
    nc.compile()
    return nc


def _get_nc():
    global _nc_cache
    if _nc_cache is None:
        _nc_cache = _build()
    return _nc_cache


def _run(x, **spmd_kwargs):
    x = np.ascontiguousarray(np.asarray(x, dtype=np.float32))
    assert x.shape == (B, C, T, H, W)
    nc = _get_nc()
    in_maps = [
        {"x": x[i * B_PER:(i + 1) * B_PER].reshape(ROWS, T, HW)}
        for i in range(NCORES)
    ]
    res = run_bass_kernel_spmd(nc, in_maps, list(range(NCORES)), **spmd_kwargs)
    out = np.concatenate(
        [
            np.asarray(r["s"]).astype(np.float32).reshape(B_PER, C, T, H, W)
            for r in res.results
        ],
        axis=0,
    )
    return out, res


def kernel(x):
    out, _ = _run(x)
    return out


# revision 10
# speedup vs baseline: 5.5777x; 1.0401x over previous
"""LIF spiking-neuron (soft reset) Bass kernel for Trainium2, 8-core SPMD.

Input  x: [B=32, C=128, T=16, H=32, W=32] f32
Output s: same shape, spikes in {0, 1}.

Recurrence per element over T:
    m' = z * 0.75 + x_t              (integrate; z = post-reset membrane)
    s_t = (m' > 0.5)                 (spike)
    z   = m' - 0.5 * s_t             (soft reset)

Engine assignment (the point of this design): on TRN2, DVE and GpSimd
arbitrate an exclusive shared SBUF port pair, so GpSimd buys nothing while
DVE runs 2-src ops back-to-back; and every op here except the spike compare
is unavailable or slow elsewhere. The schedule keeps only two ops per
element on the DVE and farms the reset out to the private-port engines:

    DVE : m'_g(SBUF) = stt(z_g(PSUM) * beta + x_t,g)         ~1.2 us
    DVE : s_g(SBUF)  = (m'_g > 0.5) -> fp8e4 {0,1}           ~0.7 us
    ACT : copy m'_g -> z_g (PSUM overwrite; private ports)   ~1.1 us
    PE  : z_g += (-0.5 I_fp8) @ s_g  (matmul accumulate onto the ACT-written
          value; start=False adds unconditionally once the bank's per-element
          has_written bits are set - the t=0 zero-weight pass does that)
    DMA : s shipped as raw fp8 bytes (1B/elem); host maps to f32 {0,1}

All f32 roundings match the reference order bit-for-bit (the fp8 matmul
products are 1.0*-0.5 or 0.0, and m' - 0.5*s is exactly representable), so
rel err is 0. The spike tile doubles as the PE operand and the DMA output.

x loads and s stores are coalesced two timesteps per DMA: the DRAM layout
[row, t, f] makes a 2-step slice 8KB-contiguous per row, so transfers are
half as many with double the line length. The first load covers only t=0 to
keep the startup ramp short.

Sharding: batch dim split across 8 cores (4 per core); per core the shard is
[512 (b*c) rows, 16 t, 1024 hw], rows mapped to SBUF partitions in 4 groups
of 128. z lives in PSUM: 4 groups x 4KB = all 8 banks.
"""

import numpy as np

import concourse.bacc as bacc
import concourse.mybir as mybir
import concourse.tile as tile
from concourse.bass_utils import run_bass_kernel_spmd

B, C, T, H, W = 32, 128, 16, 32, 32
NCORES = 8
B_PER = B // NCORES          # 4
ROWS = B_PER * C             # 512
HW = H * W                   # 1024
P = 128
NG = ROWS // P               # 4 partition groups
BETA = 0.75
THRESH = 0.5

F32 = mybir.dt.float32
FP8 = mybir.dt.float8e4
ALU = mybir.AluOpType

_nc_cache = None


def _build():
    import ml_dtypes

    nc = bacc.Bacc(
        "TRN2",
        target_bir_lowering=False,
        debug=False,
        enable_asserts=False,
        num_devices=NCORES,
    )
    x_d = nc.dram_tensor("x", [ROWS, T, HW], F32, kind="ExternalInput").ap()
    s_d = nc.dram_tensor("s", [ROWS, T, HW], FP8, kind="ExternalOutput").ap()

    x_v = x_d.rearrange("(g p) t f -> g p t f", p=P)
    s_v = s_d.rearrange("(g p) t f -> g p t f", p=P)

    wn8_d = nc.inline_tensor(
        (-THRESH * np.eye(P)).astype(ml_dtypes.float8_e4m3fn), name="wn8")
    wz_d = nc.inline_tensor(
        np.zeros((P, P), dtype=ml_dtypes.float8_e4m3fn), name="wz")

    HALF = HW // 2  # one PSUM bank of fp32

    # time blocks: [0], [1,2], [3,4], ..., [15]
    blocks = [[0]] + [[t, t + 1] for t in range(1, T - 1, 2)] + [[T - 1]]

    with tile.TileContext(nc) as tc:
        with (
            tc.tile_pool(name="mp", bufs=1) as m_pool,
            tc.tile_pool(name="xp", bufs=3) as x_pool,
            tc.tile_pool(name="sp", bufs=3) as s_pool,
            tc.tile_pool(name="wp", bufs=1) as w_pool,
            tc.tile_pool(name="zp", bufs=1, space="PSUM") as z_pool,
        ):
            wn8 = w_pool.tile([P, P], FP8, tag="wn8", name="wn8")
            wz = w_pool.tile([P, P], FP8, tag="wz", name="wz")
            nc.sync.dma_start(wn8[:], wn8_d.ap()[:])
            nc.sync.dma_start(wz[:], wz_d.ap()[:])

            z_tiles = [
                z_pool.tile([P, HW], F32, tag=f"z{g}", name=f"z{g}")
                for g in range(NG)
            ]
            m_tiles = [
                m_pool.tile([P, HW], F32, tag=f"m{g}", name=f"m{g}")
                for g in range(NG)
            ]

            def load_block(blk):
                # [P, NG, len(blk)*HW]; the (t f) merge keeps the DMA AP 3-dim
                # and makes DRAM lines 8KB-contiguous when len(blk) == 2
                tb = len(blk)
                xt = x_pool.tile([P, NG, tb * HW], F32, tag="xt", name="xt")
                nc.sync.dma_start(
                    xt[:],
                    x_v[:, :, blk[0]:blk[0] + tb, :].rearrange(
                        "g p t f -> p g (t f)"),
                )
                return xt

            def store_block(blk, st):
                tb = len(blk)
                nc.scalar.dma_start(
                    s_v[:, :, blk[0]:blk[0] + tb, :].rearrange(
                        "g p t f -> p g (t f)"),
                    st[:],
                )

            x_blk = load_block(blocks[0])
            for bi, blk in enumerate(blocks):
                x_next = load_block(blocks[bi + 1]) if bi + 1 < len(blocks) \
                    else None
                st = s_pool.tile([P, NG, len(blk) * HW], FP8, tag="st",
                                 name="st")
                for ti, t in enumerate(blk):
                    tf = slice(ti * HW, (ti + 1) * HW)
                    srcs = []
                    for g in range(NG):
                        if t == 0:
                            srcs.append(x_blk[:, g, tf])
                        else:
                            nc.vector.scalar_tensor_tensor(
                                m_tiles[g][:], z_tiles[g][:], BETA,
                                x_blk[:, g, tf],
                                op0=ALU.mult, op1=ALU.add,
                            )
                            srcs.append(m_tiles[g][:])
                    for g in range(NG):
                        nc.vector.tensor_scalar(
                            st[:, g, tf], srcs[g], THRESH, None, ALU.is_gt)
                    if t < T - 1:
                        for g in range(NG):
                            if t == 0:
                                # A start=False matmul on a bank whose
                                # per-element has_written bits are clear
                                # OVERWRITES instead of accumulating. Set the
                                # bits with a 0-weight pass before the copy.
                                for c in range(2):
                                    cols = slice(ti * HW + c * HALF,
                                                 ti * HW + (c + 1) * HALF)
                                    nc.tensor.matmul(
                                        z_tiles[g][:, c * HALF:(c + 1) * HALF],
                                        wz[:], st[:, g, cols],
                                        start=True, stop=False,
                                        skip_group_check=True,
                                    )
                            nc.scalar.copy(z_tiles[g][:], srcs[g])
                            for c in range(2):
                                cols = slice(ti * HW + c * HALF,
                                             ti * HW + (c + 1) * HALF)
                                nc.tensor.matmul(
                                    z_tiles[g][:, c * HALF:(c + 1) * HALF],
                                    wn8[:], st[:, g, cols],
                                    start=False, stop=True,
                                    skip_group_check=True,
                                )
                store_block(blk, st)
                x_blk = x_next
    nc.compile()
    return nc


def _get_nc():
    global _nc_cache
    if _nc_cache is None:
        _nc_cache = _build()
    return _nc_cache


def _run(x, **spmd_kwargs):
    x = np.ascontiguousarray(np.asarray(x, dtype=np.float32))
    assert x.shape == (B, C, T, H, W)
    nc = _get_nc()
    in_maps = [
        {"x": x[i * B_PER:(i + 1) * B_PER].reshape(ROWS, T, HW)}
        for i in range(NCORES)
    ]
    res = run_bass_kernel_spmd(nc, in_maps, list(range(NCORES)), **spmd_kwargs)
    out = np.concatenate(
        [
            np.asarray(r["s"]).astype(np.float32).reshape(B_PER, C, T, H, W)
            for r in res.results
        ],
        axis=0,
    )
    return out, res


def kernel(x):
    out, _ = _run(x)
    return out
